# revision 19
# baseline (speedup 1.0000x reference)
"""GQA attention (B=2,T=2048,D=1024,H=16,Hkv=4) on 8 trn2 NeuronCores.

Tunnel-optimized: the axon host<->device link dominates wall time, so all
large transfers are f16 and nothing is replicated on the wire:

  core = b*4 + g  (b = batch, g = kv-head group)
  - one packed f16 upload per core: x T-shard [512,1024] + half of group g's
    packed weights (WqT|WkvT|WoT). AllGather(4) rebuilds x[b]; AllGather(2)
    across the batch pair rebuilds the weights.
  - rope tables (f16, 64 rows) sharded 8-way, AllGather(8); cast to f32 and
    row-duplicated on device. identity/causal-mask/ones built on device
    (memset/affine_select).
  - each core computes its 4 q-heads' attention + partial out^T; a f32
    ReduceScatter(4) sums partials on device; each core downloads only its
    [256,2048] j-slice, int8-quantized with per-(feature, 16-col) scales
    shipped in-band as f16 bytes in ONE output tensor (each extra output
    tensor costs ~70ms/call in the PJRT donate+fetch path).
"""

import os
import sys
import math

import numpy as np
import ml_dtypes

sys.path.insert(0, "/opt/trn_rl_repo")

import jax

try:
    os.makedirs("/tmp/jax_comp_cache", exist_ok=True)
    jax.config.update("jax_compilation_cache_dir", "/tmp/jax_comp_cache")
    jax.config.update("jax_persistent_cache_min_compile_time_secs", 0.0)
    jax.config.update("jax_persistent_cache_min_entry_size_bytes", 0)
except Exception:
    pass  # cache is an optimization only

import concourse.bass as bass
import concourse.bacc as bacc
import concourse.mybir as mybir
import concourse.tile as tile
from concourse.bass_utils import run_bass_kernel_spmd

B, T, D = 2, 2048, 1024
H, HKV, DH = 16, 4, 64
NQ = H // HKV            # 4 q heads per core
GDIM = NQ * DH           # 256 local q dims per core
P = 128
NKT = D // P             # 8 contract tiles for projections
NKC = T // P             # 16 key chunks
NTC = T // 512           # 4 col chunks of 512
F32 = mybir.dt.float32
F32R = mybir.dt.float32r
BF16 = mybir.dt.bfloat16
F16 = mybir.dt.float16
I8 = mybir.dt.int8
SCALE = 1.0 / math.sqrt(DH)
MASKVAL = -30000.0
BF = np.float16

NX = (T // 4) * D        # 524288 x-shard elems per core
NWQ = D * GDIM           # 262144 elems in wqT
NWKV = D * P             # 131072 elems in wkvT
NWO = GDIM * D           # 262144 elems in woT
NW = NWQ + NWKV + NWO    # 655360 packed weight elems per group
NXW = NX + NW // 2       # 851968 packed upload elems per core
NROPE = DH * T           # 131072 elems per (64-row) rope table

G4 = [[0, 1, 2, 3], [4, 5, 6, 7]]
G2 = [[0, 4], [1, 5], [2, 6], [3, 7]]
G8 = [[0, 1, 2, 3, 4, 5, 6, 7]]

_CACHE = {}


def _build():
    nc = bacc.Bacc("TRN2", target_bir_lowering=False, debug=False, num_devices=8)

    xw_d = nc.dram_tensor("xw", [NXW], F16, kind="ExternalInput")
    rp_d = nc.dram_tensor("rp", [2 * NROPE // 8], F16, kind="ExternalInput")
    # one output row = 2048 int8 (16-col-block quantized) + 128 f16 scales
    outS_d = nc.dram_tensor("outS", [GDIM, T + 2 * P], I8, kind="ExternalOutput")

    with tile.TileContext(nc) as tc:
        with tc.tile_pool(name="dram", bufs=1, space="DRAM") as dram:
            XB = dram.tile([NX], F16, name="XB")
            WSB = dram.tile([NW // 2], F16, name="WSB")
            RSB = dram.tile([2 * NROPE // 8], F16, name="RSB")
            XG = dram.tile([4 * NX], F16, name="XG")
            WPK = dram.tile([NW], F16, name="WPK")
            ROPE = dram.tile([2 * NROPE], F16, name="ROPE")
            OUTP = dram.tile([D, T], F32, name="OUTP")
            RSO = dram.tile([GDIM, T], F32, name="RSO")

            nc.sync.dma_start(XB[:], xw_d.ap()[0:NX])
            nc.sync.dma_start(WSB[:], xw_d.ap()[NX:NXW])
            nc.sync.dma_start(RSB[:], rp_d.ap())
            nc.gpsimd.collective_compute(
                "AllGather", mybir.AluOpType.bypass, replica_groups=G4,
                ins=[XB.opt()], outs=[XG.opt()],
            )
            nc.gpsimd.collective_compute(
                "AllGather", mybir.AluOpType.bypass, replica_groups=G2,
                ins=[WSB.opt()], outs=[WPK.opt()],
            )
            nc.gpsimd.collective_compute(
                "AllGather", mybir.AluOpType.bypass, replica_groups=G8,
                ins=[RSB.opt()], outs=[ROPE.opt()],
            )

            with tc.tile_pool(name="persist", bufs=1) as pp:
                wq_sb = pp.tile([P, NKT, GDIM], F16, name="wq_sb")
                wkv_sb = pp.tile([P, NKT, P], F16, name="wkv_sb")
                wo_sb = pp.tile([P, 2, D], F16, name="wo_sb")
                wof_sb = pp.tile([P, 2, D], F32R, name="wof_sb")
                ropeC_sb = pp.tile([P, T], F32, name="ropeC_sb")
                ropeS_sb = pp.tile([P, T], F32, name="ropeS_sb")
                identB_sb = pp.tile([P, P], F16, name="identB_sb")
                maskT_sb = pp.tile([P, P], F16, name="maskT_sb")
                identD_sb = pp.tile([P, DH], F32R, name="identD_sb")
                ones_sb = pp.tile([P, DH], F32, name="ones_sb")
                qt0 = pp.tile([P, T], F32R, name="qt0")
                qt1 = pp.tile([P, T], F32R, name="qt1")
                kvt = pp.tile([P, T], F32R, name="kvt")
                k2 = pp.tile([P, T], F32R, name="k2")
                vaugA = pp.tile([P, NKC, P], F32R, name="vaugA")
                vaugB = pp.tile([P, NKC, P], F32R, name="vaugB")
                pt = pp.tile([P, T], F32R, name="pt")
                yt0 = pp.tile([P, T], F32R, name="yt0")
                yt1 = pp.tile([P, T], F32R, name="yt1")

                # ----- on-device constants (f32 staging: memset can't write
                # f32r, and affine_select is only validated on f32 here) -----
                nc.vector.memset(ones_sb[:], 1.0)
                with tc.tile_pool(name="cstage", bufs=1) as cst:
                    sIB = cst.tile([P, P], F32, name="sIB")
                    sMT = cst.tile([P, P], F32, name="sMT")
                    sID = cst.tile([P, DH], F32, name="sID")
                    nc.vector.memset(sIB[:], 0.0)
                    nc.gpsimd.affine_select(
                        out=sIB[:], in_=sIB[:],
                        compare_op=mybir.AluOpType.not_equal, fill=1.0,
                        base=0, pattern=[[-1, P]], channel_multiplier=1,
                    )
                    nc.vector.memset(sMT[:], 0.0)
                    nc.gpsimd.affine_select(
                        out=sMT[:], in_=sMT[:],
                        compare_op=mybir.AluOpType.is_ge, fill=MASKVAL,
                        base=0, pattern=[[-1, P]], channel_multiplier=1,
                    )
                    # eye(64) on partitions 64:128 (rows 0:64 unused)
                    nc.vector.memset(sID[:], 0.0)
                    nc.gpsimd.affine_select(
                        out=sID[:], in_=sID[:],
                        compare_op=mybir.AluOpType.not_equal, fill=1.0,
                        base=-64, pattern=[[-1, DH]], channel_multiplier=1,
                    )
                    nc.vector.tensor_copy(identB_sb[:], sIB[:])
                    nc.vector.tensor_copy(maskT_sb[:], sMT[:])
                    nc.vector.tensor_copy(identD_sb[:], sID[:])

                nc.sync.dma_start(
                    wq_sb[:], WPK.opt()[0:NWQ].rearrange("(o p m) -> p o m", p=P, m=GDIM)
                )
                nc.sync.dma_start(
                    wkv_sb[:],
                    WPK.opt()[NWQ:NWQ + NWKV].rearrange("(o p m) -> p o m", p=P, m=P),
                )
                nc.sync.dma_start(
                    wo_sb[:],
                    WPK.opt()[NWQ + NWKV:NW].rearrange("(c p j) -> p c j", p=P, j=D),
                )
                nc.vector.tensor_copy(wof_sb[:], wo_sb[:])
                with tc.tile_pool(name="rstage", bufs=1) as rst:
                    ropeH = rst.tile([DH, 2, T], F16, name="ropeH")
                    nc.sync.dma_start(
                        ropeH[:], ROPE.opt().rearrange("(c p t) -> p c t", p=DH, t=T)
                    )
                    nc.vector.tensor_copy(ropeC_sb[0:DH, :], ropeH[:, 0, :])
                    nc.vector.tensor_copy(ropeC_sb[DH:P, :], ropeH[:, 0, :])
                    nc.vector.tensor_copy(ropeS_sb[0:DH, :], ropeH[:, 1, :])
                    nc.vector.tensor_copy(ropeS_sb[DH:P, :], ropeH[:, 1, :])

                qts = [qt0, qt1]
                yts = [yt0, yt1]

                # ---------------- x transpose + projections ----------------
                with tc.tile_pool(name="xtp", bufs=1) as xtp, \
                     tc.tile_pool(name="ppsum", bufs=3, space="PSUM") as ppsum, \
                     tc.tile_pool(name="rotp", bufs=1) as rotp:
                    xt = xtp.tile([P, NKT, T], F16, name="xt")
                    with tc.tile_pool(name="xnp", bufs=1) as xnp:
                        xn = xnp.tile([P, NKC, D], F16, name="xn")
                        nc.sync.dma_start(
                            xn[:], XG.opt().rearrange("(o p d) -> p o d", p=P, d=D)
                        )
                        for dc in range(NKT):
                            for tcq in range(NTC):
                                px = ppsum.tile([P, 512], F16, tag="ppt", name="px")
                                for j in range(4):
                                    tci = tcq * 4 + j
                                    nc.tensor.transpose(
                                        px[:, j * P:(j + 1) * P],
                                        xn[:, tci, dc * P:(dc + 1) * P],
                                        identB_sb[:],
                                    )
                                nc.any.tensor_copy(
                                    xt[:, dc, tcq * 512:(tcq + 1) * 512], px[:]
                                )

                    strips = [
                        (qt0, lambda kt: wq_sb[:, kt, 0:128]),
                        (qt1, lambda kt: wq_sb[:, kt, 128:256]),
                        (kvt, lambda kt: wkv_sb[:, kt, :]),
                    ]
                    for strip, wsel in strips:
                        for tci in range(NTC):
                            ps = ppsum.tile([P, 512], F32, tag="pp", name="ps")
                            for kt in range(NKT):
                                nc.tensor.matmul(
                                    ps[:],
                                    wsel(kt),
                                    xt[:, kt, tci * 512:(tci + 1) * 512],
                                    start=(kt == 0), stop=(kt == NKT - 1),
                                )
                            nc.any.tensor_copy(strip[:, tci * 512:(tci + 1) * 512], ps[:])

                    # ---------------- rope ----------------
                    def rope(strip, nrows):
                        rotu = rotp.tile([P, T], F32R, tag="rotu", name="rotu")
                        for b0 in range(0, nrows, 64):
                            nc.sync.dma_start(rotu[b0:b0 + 32, :], strip[b0 + 32:b0 + 64, :])
                            nc.sync.dma_start(rotu[b0 + 32:b0 + 64, :], strip[b0:b0 + 32, :])
                        nc.vector.tensor_mul(strip[0:nrows, :], strip[0:nrows, :], ropeC_sb[0:nrows, :])
                        nc.vector.tensor_mul(rotu[0:nrows, :], rotu[0:nrows, :], ropeS_sb[0:nrows, :])
                        nc.vector.tensor_add(strip[0:nrows, :], strip[0:nrows, :], rotu[0:nrows, :])

                    rope(qt0, 128)
                    rope(qt1, 128)
                    rope(kvt, 64)

                    # duplicate roped K^T to partitions 64:128 for odd heads
                    nc.sync.dma_start(k2[64:128, :], kvt[0:64, :])

                    # ---------------- V natural + ones ----------------
                    nc.vector.tensor_copy(
                        vaugA[:, :, 64:128], ones_sb[:, None, :].to_broadcast((P, NKC, DH))
                    )
                    nc.vector.tensor_copy(
                        vaugB[:, :, 0:64], ones_sb[:, None, :].to_broadcast((P, NKC, DH))
                    )
                    for kc in range(NKC):
                        pv = ppsum.tile([P, 512], F32R, tag="pp", name="pv")
                        nc.tensor.transpose(
                            pv[:, 0:DH],
                            kvt[64:128, kc * P:(kc + 1) * P],
                            identD_sb[64:128, :],
                        )
                        nc.any.tensor_copy(vaugA[:, kc, 0:64], pv[:, 0:DH])
                        nc.any.tensor_copy(vaugB[:, kc, 64:128], pv[:, 0:DH])

                # ---------------- attention ----------------
                with tc.tile_pool(name="spsum", bufs=1, space="PSUM") as spsum, \
                     tc.tile_pool(name="opsum", bufs=1, space="PSUM") as opsum, \
                     tc.tile_pool(name="rcp", bufs=2) as rcp:
                    for h in range(NQ):
                        s, par = h // 2, h % 2
                        qs = qts[s]
                        ksrc, kbase = (kvt, 0) if par == 0 else (k2, 64)
                        vaug = vaugA if par == 0 else vaugB
                        obase = 0 if par == 0 else 64    # O^T rows in psum
                        sbase = 64 - obase               # sums rows in psum

                        ps_O = opsum.tile([P, T], F32, tag="O", name="ps_O")
                        for kc in range(NKC):
                            q0 = kc * P
                            qc0 = kc // 4
                            ps_S = spsum.tile([P, T], F32, tag="S", name="ps_S")
                            for qc in range(qc0, NTC):
                                c0 = max(q0, qc * 512)
                                c1 = (qc + 1) * 512
                                first = qc == qc0
                                nc.tensor.matmul(
                                    ps_S[:, c0:c1],
                                    ksrc[kbase:kbase + 64, q0:q0 + P],
                                    qs[kbase:kbase + 64, c0:c1],
                                    start=True, stop=not first,
                                )
                                if first:
                                    nc.tensor.matmul(
                                        ps_S[:, q0:q0 + P],
                                        maskT_sb[:],
                                        identB_sb[:],
                                        start=False, stop=True,
                                    )
                            nc.scalar.activation(
                                pt[:, q0:T], ps_S[:, q0:T],
                                mybir.ActivationFunctionType.Exp, scale=SCALE,
                            )
                            for qc in range(qc0, NTC):
                                c0 = max(q0, qc * 512)
                                c1 = (qc + 1) * 512
                                nc.tensor.matmul(
                                    ps_O[:, c0:c1],
                                    vaug[:, kc, :],
                                    pt[:, c0:c1],
                                    start=(kc == 0), stop=(kc == qc * 4 + 3),
                                )

                        # custom-DVE reciprocal only works at base partition 0, so
                        # stage sums at rows 0:64 of rc, recip into rc2[0:64], then
                        # broadcast rc2 to the O rows' partition range.
                        rc = rcp.tile([P, T], F32, tag="rc", name="rc")
                        rc2 = rcp.tile([P, T], F32, tag="rc2", name="rc2")
                        nc.vector.tensor_copy(
                            rc[sbase:sbase + 64, :], ps_O[sbase:sbase + 64, :]
                        )
                        if sbase != 0:
                            nc.sync.dma_start(rc[0:64, :], rc[sbase:sbase + 64, :])
                        nc.vector.reciprocal_approx_fast(
                            out=rc2[0:64, :], in_=rc[0:64, :]
                        )
                        if obase != 0:
                            nc.sync.dma_start(rc2[obase:obase + 64, :], rc2[0:64, :])
                        nc.vector.tensor_mul(
                            yts[s][obase:obase + 64, :],
                            ps_O[obase:obase + 64, :],
                            rc2[obase:obase + 64, :],
                        )

                # ---------------- Wo + on-device reduce ----------------
                with tc.tile_pool(name="wpsum", bufs=4, space="PSUM") as wpsum, \
                     tc.tile_pool(name="outp", bufs=2) as outp:
                    OUTP_r = OUTP.opt().rearrange("(o p) t -> o p t", p=P)
                    for js in range(8):
                        osb = outp.tile([P, T], F32, tag="osb", name="osb")
                        for tci in range(NTC):
                            pw = wpsum.tile([P, 512], F32, tag="wo", name="pw")
                            for ct in range(2):
                                nc.tensor.matmul(
                                    pw[:],
                                    wof_sb[:, ct, js * P:(js + 1) * P],
                                    yts[ct][:, tci * 512:(tci + 1) * 512],
                                    start=(ct == 0), stop=(ct == 1),
                                )
                            nc.any.tensor_copy(osb[:, tci * 512:(tci + 1) * 512], pw[:])
                        nc.sync.dma_start(OUTP_r[js], osb[:])

                    nc.gpsimd.collective_compute(
                        "ReduceScatter", mybir.AluOpType.add, replica_groups=G4,
                        ins=[OUTP.opt()], outs=[RSO.opt()],
                    )
                    # int8-quantize the output slice with per-(feature,
                    # 64-col-block) scales: err <= blockmax/252 <= 0.4% of the
                    # global max, like a bf16 cast, at half the bytes.
                    # int8-quantize with per-(feature, 16-col-block) scales;
                    # the f16-rounded reciprocal is used for the multiply AND
                    # shipped in-band, so host dequant is bit-consistent.
                    rso_sb = outp.tile([P, 2, P, 16], F32, tag="rso", bufs=1, name="rso_sb")
                    am = outp.tile([P, 2, P, 1], F32, tag="am", bufs=1, name="am")
                    inv = outp.tile([P, 2, P], F32, tag="inv", bufs=1, name="inv")
                    inv16 = outp.tile([P, 2, P], F16, tag="inv16", bufs=1, name="inv16")
                    inv2 = outp.tile([P, 2, P], F32, tag="inv2", bufs=1, name="inv2")
                    qf = outp.tile([P, 2, P, 16], F32, tag="qf", bufs=1, name="qf")
                    qi = outp.tile([P, 2, P, 16], I8, tag="qi", bufs=1, name="qi")
                    nc.sync.dma_start(
                        rso_sb[:],
                        RSO.opt().rearrange("(c p) (n b) -> p c n b", p=P, b=16),
                    )
                    nc.vector.tensor_reduce(
                        am[:], rso_sb[:], axis=mybir.AxisListType.X,
                        op=mybir.AluOpType.max, apply_absolute_value=True,
                    )
                    nc.vector.tensor_scalar_mul(am[:], am[:], 1.0 / 126.0)
                    nc.vector.tensor_scalar_max(am[:], am[:], 1e-30)
                    nc.vector.reciprocal_approx_fast(out=inv[:], in_=am[:, :, :, 0])
                    nc.vector.tensor_copy(inv16[:], inv[:])
                    nc.vector.tensor_copy(inv2[:], inv16[:])
                    nc.vector.tensor_mul(
                        qf[:], rso_sb[:],
                        inv2[:, :, :, None].to_broadcast((P, 2, P, 16)),
                    )
                    nc.any.tensor_copy(qi[:], qf[:])
                    nc.sync.dma_start(
                        outS_d.ap()[:, 0:T].rearrange("(c p) (n b) -> p c n b", p=P, b=16),
                        qi[:],
                    )
                    nc.sync.dma_start(
                        outS_d.ap()[:, T:T + 2 * P].rearrange("(c p) s -> p c s", p=P),
                        inv16[:].bitcast(I8),
                    )
    nc.finalize()
    return nc


def _host_inputs(x, rope_cos, rope_sin, Wq, Wk, Wv, Wo):
    # rope tables cast straight into the f16 upload buffer (no f32 temps);
    # sin rows 0:32 negated for the rotate-half trick
    ropeflat = np.empty(2 * NROPE, dtype=np.float16)
    rf = ropeflat.reshape(2, DH, T)
    rf[0] = rope_cos[0, 0].T
    sinT = rope_sin[0, 0].T
    rf[1, 0:32] = -sinT[0:32]
    rf[1, 32:DH] = sinT[32:DH]
    wpacks = []
    for g in range(HKV):
        wqT = Wq[g * GDIM:(g + 1) * GDIM, :].T.astype(BF)       # [1024,256]
        wkv = np.concatenate([Wk[g * DH:(g + 1) * DH], Wv[g * DH:(g + 1) * DH]], axis=0)
        wkvT = wkv.T.astype(BF)                                  # [1024,128]
        woT = Wo[:, g * GDIM:(g + 1) * GDIM].T.astype(BF)        # [256,1024]
        wpacks.append(np.concatenate([wqT.ravel(), wkvT.ravel(), woT.ravel()]))

    nw2, nr8 = NW // 2, 2 * NROPE // 8
    in_maps = []
    for core in range(8):
        b, g = core // HKV, core % HKV
        xw = np.empty(NXW, dtype=BF)
        # assignment casts f32 -> f16 in one pass, no intermediate buffer
        xw[0:NX].reshape(T // 4, D)[:] = x[b, (T // 4) * g:(T // 4) * (g + 1), :]
        xw[NX:] = wpacks[g][b * nw2:(b + 1) * nw2]
        in_maps.append({
            "xw": xw,
            "rp": ropeflat[core * nr8:(core + 1) * nr8],
        })
    return in_maps


LAST_RESULTS = None


def kernel(x, rope_cos, rope_sin, attn_mask, Wq, Wk, Wv, Wo):
    global LAST_RESULTS
    if "nc" not in _CACHE:
        _CACHE["nc"] = _build()
    nc = _CACHE["nc"]
    in_maps = _host_inputs(
        np.asarray(x), np.asarray(rope_cos), np.asarray(rope_sin),
        np.asarray(Wq), np.asarray(Wk), np.asarray(Wv), np.asarray(Wo),
    )
    res = run_bass_kernel_spmd(nc, in_maps, core_ids=list(range(8)))
    LAST_RESULTS = res
    out = np.empty((B, T, D), dtype=np.float32)
    for b in range(B):
        for g in range(HKV):
            buf = res.results[b * HKV + g]["outS"]
            q = buf[:, :T].reshape(GDIM, P, 16)      # int8, 16-col blocks
            inv16 = buf[:, T:].copy().view(np.float16)     # [256, 128]
            s = 1.0 / inv16.astype(np.float32)
            deq = (q * s[:, :, None]).reshape(GDIM, T)
            out[b, :, g * GDIM:(g + 1) * GDIM] = deq.T
    return out
